# revision 1
# baseline (speedup 1.0000x reference)
"""Trainium2 Bass kernel for an AttentionBlock (GroupNorm -> q/k/v 1x1 conv ->
full S x S attention -> proj 1x1 conv -> residual).

Problem shapes: x [4, 512, 64, 64] fp32, S = 4096 tokens, C = 512 channels,
GroupNorm with 32 groups of 16 channels.

Sharding: 8 cores = 4 batches x 2 query-halves. Core c handles batch c//2 and
query rows [half*2048, (half+1)*2048). Each core of a batch-pair redundantly
computes k/v for its batch (cheap vs attention) so no collectives are needed.

Math optimizations baked in:
  * GroupNorm is folded into the q/k/v weights: h = scale_c * x + shift_c with
    per-channel scale/shift derived from group stats, so
    q = (wq * scale) @ x + (bq + wq @ shift), and similarly k, v.
  * k's bias term (bk + wk @ shift) adds a per-query constant to every softmax
    row and cancels exactly -> never computed (bk unused).
  * v's bias adds bv' * sum_j(attn) = bv' to the attention output (softmax rows
    sum to 1), which is then folded into the proj bias:
    bp' = bp + wp @ (bv + wv @ shift).
  * Softmax is computed without max-subtraction (scores are O(10) here, exp is
    safe in fp32), with the denominator accumulated by a ones-vector matmul.

Dtypes: big matmuls run in float32r (full PE speed, ~13-bit mantissa);
attention probabilities and v^T are bf16 (their error is diluted by the
residual connection); everything else fp32.

Layouts per core (partition dim first):
  q  [c=512, i=2048] f32r   (4 tiles [128, 2048])    scores rhs
  k  [c=512, j=4096] f32r   (4 tiles [128, 4096])    scores lhsT (stationary)
  vT [j=4096, c=512] bf16   (1 tile [128, 32, 512])  attn@v lhsT
  scores^T [j, i] so softmax reduction over j uses matmul tricks; attention
  output lands as h [c, i] which feeds proj directly.
"""

import numpy as np
import ml_dtypes

import concourse.bacc as bacc
import concourse.tile as tile
from concourse import mybir
from concourse.bass_utils import run_bass_kernel_spmd

F32 = mybir.dt.float32
F32R = mybir.dt.float32r
BF16 = mybir.dt.bfloat16
AF = mybir.ActivationFunctionType
OP = mybir.AluOpType
AX = mybir.AxisListType

C = 512
S = 4096
B = 4
NCORES = 8
CT = 4          # channel tiles of 128
SBLK = 8        # s-blocks of 512 for k/v/stats
QBLK = 4        # q-blocks of 512 (half = 2048 columns)
IB = 4          # i-blocks of 512 for attention
IBW = 512
JT = 32         # j-tiles of 128
HALF = S // 2
EPS = 1e-5
GELEMS = 16 * S                      # elements per group (16 ch x 4096)
SCL = 1.0 / np.sqrt(np.float32(C))   # softmax scale


def build_nc(reps=1):
    """Build and compile the SPMD single-core program."""
    nc = bacc.Bacc("TRN2", target_bir_lowering=False, debug=False,
                   num_devices=NCORES)

    x_d = nc.dram_tensor("x", [CT, 128, S], F32R, kind="ExternalInput").ap()
    wqt_d = nc.dram_tensor("wqt", [CT, 128, C], F32R, kind="ExternalInput").ap()
    wkt_d = nc.dram_tensor("wkt", [CT, 128, C], F32R, kind="ExternalInput").ap()
    wvt_d = nc.dram_tensor("wvt", [CT, 128, C], F32R, kind="ExternalInput").ap()
    wpt_d = nc.dram_tensor("wpt", [CT, 128, C], F32R, kind="ExternalInput").ap()
    bq_d = nc.dram_tensor("bq", [CT, 128, 1], F32, kind="ExternalInput").ap()
    bv_d = nc.dram_tensor("bv", [CT, 128, 1], F32, kind="ExternalInput").ap()
    bp_d = nc.dram_tensor("bp", [CT, 128, 1], F32, kind="ExternalInput").ap()
    gnw_d = nc.dram_tensor("gnw", [CT, 128, 1], F32, kind="ExternalInput").ap()
    gnb_d = nc.dram_tensor("gnb", [CT, 128, 1], F32, kind="ExternalInput").ap()
    g16_d = nc.dram_tensor("g16", [128, 8], F32, kind="ExternalInput").ap()
    b8_d = nc.dram_tensor("b8", [8, 128], F32, kind="ExternalInput").ap()
    onbf_d = nc.dram_tensor("onbf", [128, 128], F32R, kind="ExternalInput").ap()
    out_d = nc.dram_tensor("out", [CT, 128, HALF], F32, kind="ExternalOutput").ap()

    with tile.TileContext(nc) as tc:
        with tc.tile_pool(name="const", bufs=1) as cpool, \
             tc.tile_pool(name="resident", bufs=1) as rpool:
            # constants loaded once
            g16_t = cpool.tile([128, 8], F32, name="g16t")
            b8_t = cpool.tile([8, 128], F32, name="b8t")
            onbf_t = cpool.tile([128, 128], F32R, name="onbft")
            eps_t = cpool.tile([8, 1], F32, name="epst")
            nc.sync.dma_start(g16_t[:], g16_d[:])
            nc.sync.dma_start(b8_t[:], b8_d[:])
            nc.sync.dma_start(onbf_t[:], onbf_d[:])
            nc.vector.memset(eps_t[:], EPS)
            gnw_t, gnb_t = [], []
            for ci in range(CT):
                gw = cpool.tile([128, 1], F32, name=f"gnw{ci}")
                gb = cpool.tile([128, 1], F32, name=f"gnb{ci}")
                nc.sync.dma_start(gw[:], gnw_d[ci])
                nc.sync.dma_start(gb[:], gnb_d[ci])
                gnw_t.append(gw)
                gnb_t.append(gb)

            for rep in range(reps):
                emit_rep(nc, tc, rpool, rep,
                         x_d, wqt_d, wkt_d, wvt_d, wpt_d,
                         bq_d, bv_d, bp_d,
                         g16_t, b8_t, onbf_t, eps_t, gnw_t, gnb_t,
                         out_d)
    nc.compile()
    return nc


def emit_rep(nc, tc, rpool, rep, x_d, wqt_d, wkt_d, wvt_d, wpt_d,
             bq_d, bv_d, bp_d, g16_t, b8_t, onbf_t, eps_t,
             gnw_t, gnb_t, out_d):
    # ---- resident tensors (slots shared across reps via fixed tags) ----
    k_sb = [rpool.tile([128, S], BF16, name=f"k{ci}_{rep}", tag=f"k{ci}")
            for ci in range(CT)]
    q_sb = [rpool.tile([128, HALF], BF16, name=f"q{ci}_{rep}", tag=f"q{ci}")
            for ci in range(CT)]
    vT_sb = rpool.tile([128, JT, C], BF16, name=f"vT_{rep}", tag="vT")
    wpt_s = [rpool.tile([128, C], F32R, name=f"wp{ci}_{rep}", tag=f"wp{ci}")
             for ci in range(CT)]
    for ci in range(CT):
        nc.sync.dma_start(wpt_s[ci][:], wpt_d[ci])

    with tc.tile_pool(name=f"xblk_{rep}", bufs=2) as xpool, \
         tc.tile_pool(name=f"stat_{rep}", bufs=1) as spool, \
         tc.tile_pool(name=f"sscr_{rep}", bufs=2) as scrpool, \
         tc.tile_pool(name=f"pstat_{rep}", bufs=1, space="PSUM") as pstats:

        # ================= P1: per-channel sum / sumsq over x =================
        sums = spool.tile([128, CT, SBLK], F32, name=f"sums_{rep}", tag="sums")
        sumsq = spool.tile([128, CT, SBLK], F32, name=f"sumsq_{rep}", tag="sumsq")
        for sb in range(SBLK):
            xb = [xpool.tile([128, 512], F32R, name=f"xa{sb}_{ci}_{rep}", tag=f"xb{ci}")
                  for ci in range(CT)]
            for ci in range(CT):
                nc.sync.dma_start(xb[ci][:], x_d[ci, :, sb * 512:(sb + 1) * 512])
            for ci in range(CT):
                xf = xb[ci][:].bitcast(F32)
                nc.vector.reduce_sum(out=sums[:, ci, sb:sb + 1], in_=xf, axis=AX.X)
                sq = scrpool.tile([128, 512], F32, name=f"sq{sb}_{ci}_{rep}", tag="sqscr")
                nc.scalar.activation(out=sq[:], in_=xf, func=AF.Square,
                                     accum_out=sumsq[:, ci, sb:sb + 1])

        # ================= P2: group stats -> per-channel scale/shift =========
        sq2 = spool.tile([128, CT, 2], F32, name=f"sq2_{rep}", tag="sq2")
        for ci in range(CT):
            nc.vector.reduce_sum(out=sq2[:, ci, 0:1], in_=sums[:, ci, :], axis=AX.X)
            nc.vector.reduce_sum(out=sq2[:, ci, 1:2], in_=sumsq[:, ci, :], axis=AX.X)
        gpsum = pstats.tile([8, 8], F32, name=f"gps_{rep}", tag="g")
        for ci in range(CT):
            nc.tensor.matmul(gpsum[:, 2 * ci:2 * ci + 2], g16_t[:], sq2[:, ci, :],
                             start=True, stop=True)
        gp3 = gpsum[:].rearrange("p (c t) -> p c t", t=2)
        packbuf = spool.tile([8, CT, 2], F32, name=f"pack_{rep}", tag="pack")
        ex2 = spool.tile([8, CT], F32, name=f"ex2_{rep}", tag="ex2")
        gm2 = spool.tile([8, CT], F32, name=f"gm2_{rep}", tag="gm2")
        gvar = spool.tile([8, CT], F32, name=f"gvar_{rep}", tag="gvar")
        nc.scalar.mul(out=packbuf[:, :, 1], in_=gp3[:, :, 0], mul=1.0 / GELEMS)
        nc.scalar.mul(out=ex2[:], in_=gp3[:, :, 1], mul=1.0 / GELEMS)
        nc.vector.tensor_mul(gm2[:], packbuf[:, :, 1], packbuf[:, :, 1])
        nc.vector.tensor_sub(gvar[:], ex2[:], gm2[:])
        nc.scalar.activation(out=gvar[:], in_=gvar[:], func=AF.Sqrt,
                             bias=eps_t[:], scale=1.0)
        nc.vector.reciprocal(out=packbuf[:, :, 0], in_=gvar[:])
        scale_t, shift_t = [], []
        for ci in range(CT):
            bca = pstats.tile([128, 2], F32, name=f"bca{ci}_{rep}", tag="bca")
            nc.tensor.matmul(bca[:], b8_t[:], packbuf[:, ci, :], start=True, stop=True)
            sc = spool.tile([128, 1], F32, name=f"scale{ci}_{rep}", tag=f"scale{ci}")
            sh = spool.tile([128, 1], F32, name=f"shift{ci}_{rep}", tag=f"shift{ci}")
            tm = spool.tile([128, 1], F32, name=f"tmpm{ci}_{rep}", tag="tmpm")
            nc.vector.tensor_mul(sc[:], gnw_t[ci][:], bca[:, 0:1])
            nc.vector.tensor_mul(tm[:], bca[:, 1:2], sc[:])
            nc.vector.tensor_sub(sh[:], gnb_t[ci][:], tm[:])
            scale_t.append(sc)
            shift_t.append(sh)

        # ================= P3: fold GN into weights + bias folds ==============
        with tc.tile_pool(name=f"wfold_{rep}", bufs=1) as wfold:
            wq_s, wk_s, wv_s = [], [], []
            for nm, src, lst in (("wq", wqt_d, wq_s), ("wk", wkt_d, wk_s),
                                 ("wv", wvt_d, wv_s)):
                for ci in range(CT):
                    w = wfold.tile([128, C], F32R, name=f"{nm}{ci}_{rep}",
                                   tag=f"{nm}{ci}")
                    nc.sync.dma_start(w[:], src[ci])
                    lst.append(w)
            # bias folds with RAW weights: b' = b + w^T @ shift
            bq_sb, bv_sb = [], []
            for w_s, b_dram, lst, nm in ((wq_s, bq_d, bq_sb, "bq"),
                                         (wv_s, bv_d, bv_sb, "bv")):
                for co in range(CT):
                    pb = pstats.tile([128, 1], F32, name=f"pb{nm}{co}_{rep}", tag="pb")
                    for ci in range(CT):
                        nc.tensor.matmul(
                            pb[:],
                            w_s[ci][:].bitcast(F32)[:, co * 128:(co + 1) * 128],
                            shift_t[ci][:], start=(ci == 0), stop=(ci == CT - 1))
                    braw = spool.tile([128, 1], F32, name=f"{nm}r{co}_{rep}", tag="braw")
                    nc.sync.dma_start(braw[:], b_dram[co])
                    bt = spool.tile([128, 1], F32, name=f"{nm}f{co}_{rep}",
                                    tag=f"{nm}f{co}")
                    nc.vector.tensor_add(bt[:], pb[:], braw[:])
                    lst.append(bt)
            # bp' = bp + wp^T @ bv'
            bp_sb = []
            for co in range(CT):
                pb = pstats.tile([128, 1], F32, name=f"pbbp{co}_{rep}", tag="pb")
                for ci in range(CT):
                    nc.tensor.matmul(
                        pb[:], wpt_s[ci][:].bitcast(F32)[:, co * 128:(co + 1) * 128],
                        bv_sb[ci][:], start=(ci == 0), stop=(ci == CT - 1))
                braw = spool.tile([128, 1], F32, name=f"bpr{co}_{rep}", tag="braw")
                nc.sync.dma_start(braw[:], bp_d[co])
                bt = rpool.tile([128, 1], F32, name=f"bpf{co}_{rep}", tag=f"bpf{co}")
                nc.vector.tensor_add(bt[:], pb[:], braw[:])
                bp_sb.append(bt)
            # scale folds in place (f32 view in, f32r out = rounding write)
            for w_s in (wq_s, wk_s, wv_s):
                for ci in range(CT):
                    nc.vector.tensor_scalar_mul(out=w_s[ci][:],
                                                in0=w_s[ci][:].bitcast(F32),
                                                scalar1=scale_t[ci][:])

            # ================= P4: q / k / vT projections ====================
            with tc.tile_pool(name=f"pd_{rep}", bufs=5, space="PSUM") as pd:
                for sb in range(SBLK):
                    xb = [xpool.tile([128, 512], F32R, name=f"xc{sb}_{ci}_{rep}",
                                     tag=f"xb{ci}") for ci in range(CT)]
                    for ci in range(CT):
                        nc.sync.dma_start(xb[ci][:], x_d[ci, :, sb * 512:(sb + 1) * 512])
                    for co in range(CT):
                        pk = pd.tile([128, 512], F32, name=f"pk{sb}{co}_{rep}", tag="pd")
                        for ci in range(CT):
                            nc.tensor.matmul(pk[:],
                                             wk_s[ci][:, co * 128:(co + 1) * 128],
                                             xb[ci][:], start=(ci == 0),
                                             stop=(ci == CT - 1))
                        nc.vector.tensor_copy(k_sb[co][:, sb * 512:(sb + 1) * 512], pk[:])
                    for js4 in range(4):
                        pv = pd.tile([128, 512], F32, name=f"pv{sb}{js4}_{rep}", tag="pd")
                        for ci in range(CT):
                            nc.tensor.matmul(pv[:],
                                             xb[ci][:, js4 * 128:(js4 + 1) * 128],
                                             wv_s[ci][:], start=(ci == 0),
                                             stop=(ci == CT - 1))
                        nc.scalar.activation(out=vT_sb[:, sb * 4 + js4, :], in_=pv[:],
                                             func=AF.Copy)
                    if sb < QBLK:
                        # columns [0, 2048) are this core's queries (host-permuted)
                        for co in range(CT):
                            pq = pd.tile([128, 512], F32, name=f"pq{sb}{co}_{rep}", tag="pd")
                            for ci in range(CT):
                                nc.tensor.matmul(pq[:],
                                                 wq_s[ci][:, co * 128:(co + 1) * 128],
                                                 xb[ci][:], start=(ci == 0),
                                                 stop=(ci == CT - 1))
                            nc.vector.tensor_scalar(
                                out=q_sb[co][:, sb * 512:(sb + 1) * 512], in0=pq[:],
                                scalar1=bq_sb[co][:], scalar2=None, op0=OP.add)

    # ================= P5: attention + proj + residual =======================
    with tc.tile_pool(name=f"ex_{rep}", bufs=2) as expool, \
         tc.tile_pool(name=f"tsc_{rep}", bufs=1) as tscpool, \
         tc.tile_pool(name=f"hn_{rep}", bufs=1) as hnpool, \
         tc.tile_pool(name=f"eo_{rep}", bufs=3) as eopool, \
         tc.tile_pool(name=f"psc_{rep}", bufs=3, space="PSUM") as psc, \
         tc.tile_pool(name=f"pph_{rep}", bufs=4, space="PSUM") as pph, \
         tc.tile_pool(name=f"psm_{rep}", bufs=1, space="PSUM") as psm:
        for ib in range(IB):
            isl = slice(ib * IBW, (ib + 1) * IBW)
            ex = expool.tile([128, JT, IBW], BF16, name=f"ex{ib}_{rep}", tag="ex")
            for js in range(JT):
                ps_ = psc.tile([128, IBW], F32, name=f"ps{ib}{js}_{rep}", tag="ps")
                for ci in range(CT):
                    nc.tensor.matmul(ps_[:],
                                     k_sb[ci][:, js * 128:(js + 1) * 128],
                                     q_sb[ci][:, isl], start=(ci == 0),
                                     stop=(ci == CT - 1))
                nc.scalar.activation(out=ex[:, js, :], in_=ps_[:], func=AF.Exp,
                                     scale=float(SCL))
            ph = [pph.tile([128, IBW], F32, name=f"ph{ib}{ci}_{rep}", tag="ph")
                  for ci in range(CT)]
            for js in range(JT):
                for ci in range(CT):
                    nc.tensor.matmul(ph[ci][:],
                                     vT_sb[:, js, ci * 128:(ci + 1) * 128],
                                     ex[:, js, :], start=(js == 0),
                                     stop=(js == JT - 1), skip_group_check=True)
            # denominator: bf16 tree over j-tiles (DVE), then exact
            # cross-partition sum via a ones-stationary f32r matmul
            tsc = tscpool.tile([128, 8, IBW], BF16, name=f"tsc{ib}_{rep}", tag="tsc")
            den = hnpool.tile([128, IBW], F32R, name=f"den{ib}_{rep}", tag="den")
            dena = hnpool.tile([128, IBW], F32, name=f"dena{ib}_{rep}", tag="dena")
            denb = hnpool.tile([128, IBW], F32, name=f"denb{ib}_{rep}", tag="denb")
            nc.vector.tensor_add(tsc[:], ex[:, 0:8, :], ex[:, 8:16, :])
            nc.vector.tensor_add(tsc[:, 0:4, :], tsc[:, 0:4, :], tsc[:, 4:8, :])
            nc.vector.tensor_add(tsc[:, 0:2, :], tsc[:, 0:2, :], tsc[:, 2:4, :])
            nc.vector.tensor_add(dena[:], tsc[:, 0, :], tsc[:, 1, :])
            nc.vector.tensor_add(tsc[:], ex[:, 16:24, :], ex[:, 24:32, :])
            nc.vector.tensor_add(tsc[:, 0:4, :], tsc[:, 0:4, :], tsc[:, 4:8, :])
            nc.vector.tensor_add(tsc[:, 0:2, :], tsc[:, 0:2, :], tsc[:, 2:4, :])
            nc.vector.tensor_add(denb[:], tsc[:, 0, :], tsc[:, 1, :])
            nc.vector.tensor_add(den[:], dena[:], denb[:])
            pdn = psm.tile([128, IBW], F32, name=f"pdn{ib}_{rep}", tag="sm")
            nc.tensor.matmul(pdn[:], onbf_t[:], den[:], start=True, stop=True)
            rbc = hnpool.tile([128, IBW], F32, name=f"rbc{ib}_{rep}", tag="rbc")
            nc.vector.reciprocal(out=rbc[:], in_=pdn[:])
            hs = hnpool.tile([128, CT, IBW], F32R, name=f"hs{ib}_{rep}", tag="hs")
            for ci in range(CT):
                nc.vector.tensor_mul(hs[:, ci, :], ph[ci][:], rbc[:])
            for co in range(CT):
                pp = psc.tile([128, IBW], F32, name=f"pp{ib}{co}_{rep}", tag="ps")
                for ci in range(CT):
                    nc.tensor.matmul(pp[:],
                                     wpt_s[ci][:, co * 128:(co + 1) * 128],
                                     hs[:, ci, :], start=(ci == 0),
                                     stop=(ci == CT - 1))
                xr = eopool.tile([128, IBW], F32, name=f"xr{ib}{co}_{rep}", tag="xr")
                nc.sync.dma_start(xr[:], x_d[co, :, isl].bitcast(F32))
                ot = eopool.tile([128, IBW], F32, name=f"ot{ib}{co}_{rep}", tag="ot")
                nc.vector.scalar_tensor_tensor(out=ot[:], in0=pp[:],
                                               scalar=bp_sb[co][:], in1=xr[:],
                                               op0=OP.add, op1=OP.add)
                nc.sync.dma_start(out_d[co, :, isl], ot[:])


# ---------------------------------------------------------------------------
# Host side
# ---------------------------------------------------------------------------
_NC_CACHE = {}


def _get_nc(reps=1):
    if reps not in _NC_CACHE:
        _NC_CACHE[reps] = build_nc(reps)
    return _NC_CACHE[reps]


def make_in_maps(x, gn_w, gn_b, wq, bq, wk, bk, wv, bv, wp, bp):
    xf = np.ascontiguousarray(np.asarray(x, dtype=np.float32)).reshape(B, C, S)
    g16 = np.zeros((128, 8), np.float32)
    g16[np.arange(128), np.arange(128) // 16] = 1.0
    b8 = np.ascontiguousarray(g16.T)
    shared = {
        "wqt": np.ascontiguousarray(np.asarray(wq, np.float32).T).reshape(CT, 128, C),
        "wkt": np.ascontiguousarray(np.asarray(wk, np.float32).T).reshape(CT, 128, C),
        "wvt": np.ascontiguousarray(np.asarray(wv, np.float32).T).reshape(CT, 128, C),
        "wpt": np.ascontiguousarray(np.asarray(wp, np.float32).T).reshape(CT, 128, C),
        "bq": np.asarray(bq, np.float32).reshape(CT, 128, 1),
        "bv": np.asarray(bv, np.float32).reshape(CT, 128, 1),
        "bp": np.asarray(bp, np.float32).reshape(CT, 128, 1),
        "gnw": np.asarray(gn_w, np.float32).reshape(CT, 128, 1),
        "gnb": np.asarray(gn_b, np.float32).reshape(CT, 128, 1),
        "g16": g16,
        "b8": b8,
        "onbf": np.ones((128, 128), np.float32),
    }
    in_maps = []
    for core in range(NCORES):
        b, half = core // 2, core % 2
        xb = xf[b]
        if half == 0:
            xp = xb
        else:
            xp = np.concatenate([xb[:, HALF:], xb[:, :HALF]], axis=1)
        in_maps.append(dict(shared, x=np.ascontiguousarray(xp).reshape(CT, 128, S)))
    return in_maps


def assemble_out(results, H=64, W=64):
    out = np.empty((B, C, S), np.float32)
    for core in range(NCORES):
        b, half = core // 2, core % 2
        out[b][:, half * HALF:(half + 1) * HALF] = \
            results[core]["out"].reshape(C, HALF)
    return out.reshape(B, C, H, W)


def kernel(x, gn_w, gn_b, wq, bq, wk, bk, wv, bv, wp, bp, t1=64, t2=64):
    H, W = int(t1), int(t2)
    nc = _get_nc(1)
    in_maps = make_in_maps(x, gn_w, gn_b, wq, bq, wk, bk, wv, bv, wp, bp)
    res = run_bass_kernel_spmd(nc, in_maps, core_ids=list(range(NCORES)))
    return assemble_out(res.results, H, W)



# revision 6
# speedup vs baseline: 1.7664x; 1.7664x over previous
"""Trainium2 Bass kernel for an AttentionBlock (GroupNorm -> q/k/v 1x1 conv ->
full S x S attention -> proj 1x1 conv -> residual).

Problem shapes: x [4, 512, 64, 64] fp32, S = 4096 tokens, C = 512 channels,
GroupNorm with 32 groups of 16 channels.

Sharding: 8 cores = 4 batches x 2 query-halves. Core c handles batch c//2 and
query rows [half*2048, (half+1)*2048). Each core of a batch-pair redundantly
computes k/v for its batch (cheap vs attention) so no collectives are needed.

Math optimizations baked in:
  * GroupNorm is folded into the q/k/v weights: h = scale_c * x + shift_c with
    per-channel scale/shift derived from group stats, so
    q = (wq * scale) @ x + (bq + wq @ shift), and similarly k, v.
  * k's bias term (bk + wk @ shift) adds a per-query constant to every softmax
    row and cancels exactly -> never computed (bk unused).
  * v's bias adds bv' * sum_j(attn) = bv' to the attention output (softmax rows
    sum to 1), which is then folded into the proj bias:
    bp' = bp + wp @ (bv + wv @ shift).
  * Softmax is computed without max-subtraction; exp outputs are stored as
    fp8e5m2 (wide exponent range covers both the O(e^8) peaks and the diffuse
    ~1e-4-weight tail), so scores' q/k run as fp8e4m3 DoubleRow matmuls at 2x+
    the bf16 PE rate, and attn@v / the softmax denominator run as DoubleRow
    matmuls too (denominator = ones-stationary matmul accumulated in PSUM).

Dtypes: q/k/v/proj projections run in float32r (measured ~0.78 cyc/row, faster
than bf16); q/k are written as fp8e4m3, v^T as fp8e4m3, attention
probabilities as fp8e5m2; everything else fp32.

Layouts per core (partition dim first). DoubleRow operand slices MUST be
contiguous [p, 2, f] blocks (strided stationary slices crash the exec unit;
strided moving slices silently compute garbage), hence the interleaved
layouts below:
  q8  [2][128, IB, 2, 512]     e4m3   scores DR moving     (slice [:, ib])
  k8  [2][128, JT, 2, 128]     e4m3   scores DR stationary (slice [:, js])
  vT8 [128, JP, CT, 2, 128]    e4m3   attn@v DR stationary (slice [:, jp, ci])
  ex8 [128, JT, 512]           e5m2   attn@v / den DR moving ([:, 2jp:2jp+2, :])
  scores^T [j, i]; attention output lands as h [c, i] which feeds proj.
"""

import numpy as np
import ml_dtypes

import concourse.bacc as bacc
import concourse.tile as tile
from concourse import mybir
from concourse.bass_utils import run_bass_kernel_spmd

F32 = mybir.dt.float32
F32R = mybir.dt.float32r
BF16 = mybir.dt.bfloat16
FP8E4 = mybir.dt.float8e4
FP8E5 = mybir.dt.float8e5
AF = mybir.ActivationFunctionType
OP = mybir.AluOpType
AX = mybir.AxisListType
DRM = mybir.MatmulPerfMode.DoubleRow

C = 512
S = 4096
B = 4
NCORES = 8
CT = 4          # channel tiles of 128
CP = 2          # channel pairs (DoubleRow k-tiles of 256)
SBLK = 8        # s-blocks of 512 for k/v/stats
QBLK = 4        # q-blocks of 512 (half = 2048 columns)
IB = 4          # i-blocks of 512 for attention
IBW = 512
JT = 32         # j-tiles of 128
JP = 16         # j-pairs of 256 for DoubleRow
HALF = S // 2
EPS = 1e-5
GELEMS = 16 * S                      # elements per group (16 ch x 4096)
SCL = 1.0 / np.sqrt(np.float32(C))   # softmax scale


def build_nc(reps=1):
    """Build and compile the SPMD single-core program."""
    nc = bacc.Bacc("TRN2", target_bir_lowering=False, debug=False,
                   num_devices=NCORES)

    x_d = nc.dram_tensor("x", [CT, 128, S], F32R, kind="ExternalInput").ap()
    wqt_d = nc.dram_tensor("wqt", [CT, 128, C], F32R, kind="ExternalInput").ap()
    wkt_d = nc.dram_tensor("wkt", [CT, 128, C], F32R, kind="ExternalInput").ap()
    wvt_d = nc.dram_tensor("wvt", [CT, 128, C], F32R, kind="ExternalInput").ap()
    wpt_d = nc.dram_tensor("wpt", [CT, 128, C], F32R, kind="ExternalInput").ap()
    bq_d = nc.dram_tensor("bq", [CT, 128, 1], F32, kind="ExternalInput").ap()
    bv_d = nc.dram_tensor("bv", [CT, 128, 1], F32, kind="ExternalInput").ap()
    bp_d = nc.dram_tensor("bp", [CT, 128, 1], F32, kind="ExternalInput").ap()
    gnw_d = nc.dram_tensor("gnw", [CT, 128, 1], F32, kind="ExternalInput").ap()
    gnb_d = nc.dram_tensor("gnb", [CT, 128, 1], F32, kind="ExternalInput").ap()
    g16_d = nc.dram_tensor("g16", [128, 8], F32, kind="ExternalInput").ap()
    b8_d = nc.dram_tensor("b8", [8, 128], F32, kind="ExternalInput").ap()
    on8_d = nc.dram_tensor("on8", [128, 2, 128], FP8E5, kind="ExternalInput").ap()
    out_d = nc.dram_tensor("out", [CT, 128, HALF], F32, kind="ExternalOutput").ap()

    with tile.TileContext(nc) as tc:
        with tc.tile_pool(name="const", bufs=1) as cpool, \
             tc.tile_pool(name="resident", bufs=1) as rpool:
            # constants loaded once
            g16_t = cpool.tile([128, 8], F32, name="g16t")
            b8_t = cpool.tile([8, 128], F32, name="b8t")
            on8_t = cpool.tile([128, 2, 128], FP8E5, name="on8t")
            eps_t = cpool.tile([8, 1], F32, name="epst")
            nc.sync.dma_start(g16_t[:], g16_d[:])
            nc.sync.dma_start(b8_t[:], b8_d[:])
            nc.sync.dma_start(on8_t[:], on8_d[:])
            nc.vector.memset(eps_t[:], EPS)
            gnw_t, gnb_t = [], []
            for ci in range(CT):
                gw = cpool.tile([128, 1], F32, name=f"gnw{ci}")
                gb = cpool.tile([128, 1], F32, name=f"gnb{ci}")
                nc.sync.dma_start(gw[:], gnw_d[ci])
                nc.sync.dma_start(gb[:], gnb_d[ci])
                gnw_t.append(gw)
                gnb_t.append(gb)

            for rep in range(reps):
                emit_rep(nc, tc, rpool, rep,
                         x_d, wqt_d, wkt_d, wvt_d, wpt_d,
                         bq_d, bv_d, bp_d,
                         g16_t, b8_t, on8_t, eps_t, gnw_t, gnb_t,
                         out_d)
    nc.compile()
    return nc


def emit_rep(nc, tc, rpool, rep, x_d, wqt_d, wkt_d, wvt_d, wpt_d,
             bq_d, bv_d, bp_d, g16_t, b8_t, on8_t, eps_t,
             gnw_t, gnb_t, out_d):
    # ---- resident tensors (slots shared across reps via fixed tags) ----
    k8_sb = [rpool.tile([128, JT, 2, 128], FP8E4, name=f"k8{p}_{rep}", tag=f"k8{p}")
             for p in range(CP)]
    q8_sb = [rpool.tile([128, IB, 2, IBW], FP8E4, name=f"q8{p}_{rep}", tag=f"q8{p}")
             for p in range(CP)]
    vT_sb = rpool.tile([128, JP, CT, 2, 128], FP8E4, name=f"vT_{rep}", tag="vT")
    wpt_s = [rpool.tile([128, C], F32R, name=f"wp{ci}_{rep}", tag=f"wp{ci}")
             for ci in range(CT)]
    for ci in range(CT):
        nc.sync.dma_start(wpt_s[ci][:], wpt_d[ci])

    with tc.tile_pool(name=f"xblk_{rep}", bufs=2) as xpool, \
         tc.tile_pool(name=f"stat_{rep}", bufs=1) as spool, \
         tc.tile_pool(name=f"sscr_{rep}", bufs=2) as scrpool, \
         tc.tile_pool(name=f"pstat_{rep}", bufs=1, space="PSUM") as pstats:

        # ================= P1: per-channel sum / sumsq over x =================
        sums = spool.tile([128, CT, SBLK], F32, name=f"sums_{rep}", tag="sums")
        sumsq = spool.tile([128, CT, SBLK], F32, name=f"sumsq_{rep}", tag="sumsq")
        for sb in range(SBLK):
            xb = [xpool.tile([128, 512], F32R, name=f"xa{sb}_{ci}_{rep}", tag=f"xb{ci}")
                  for ci in range(CT)]
            for ci in range(CT):
                nc.sync.dma_start(xb[ci][:], x_d[ci, :, sb * 512:(sb + 1) * 512])
            for ci in range(CT):
                xf = xb[ci][:].bitcast(F32)
                nc.vector.reduce_sum(out=sums[:, ci, sb:sb + 1], in_=xf, axis=AX.X)
                sq = scrpool.tile([128, 512], F32, name=f"sq{sb}_{ci}_{rep}", tag="sqscr")
                nc.scalar.activation(out=sq[:], in_=xf, func=AF.Square,
                                     accum_out=sumsq[:, ci, sb:sb + 1])

        # ================= P2: group stats -> per-channel scale/shift =========
        sq2 = spool.tile([128, CT, 2], F32, name=f"sq2_{rep}", tag="sq2")
        for ci in range(CT):
            nc.vector.reduce_sum(out=sq2[:, ci, 0:1], in_=sums[:, ci, :], axis=AX.X)
            nc.vector.reduce_sum(out=sq2[:, ci, 1:2], in_=sumsq[:, ci, :], axis=AX.X)
        gpsum = pstats.tile([8, 8], F32, name=f"gps_{rep}", tag="g")
        for ci in range(CT):
            nc.tensor.matmul(gpsum[:, 2 * ci:2 * ci + 2], g16_t[:], sq2[:, ci, :],
                             start=True, stop=True)
        gp3 = gpsum[:].rearrange("p (c t) -> p c t", t=2)
        packbuf = spool.tile([8, CT, 2], F32, name=f"pack_{rep}", tag="pack")
        ex2 = spool.tile([8, CT], F32, name=f"ex2_{rep}", tag="ex2")
        gm2 = spool.tile([8, CT], F32, name=f"gm2_{rep}", tag="gm2")
        gvar = spool.tile([8, CT], F32, name=f"gvar_{rep}", tag="gvar")
        nc.scalar.mul(out=packbuf[:, :, 1], in_=gp3[:, :, 0], mul=1.0 / GELEMS)
        nc.scalar.mul(out=ex2[:], in_=gp3[:, :, 1], mul=1.0 / GELEMS)
        nc.vector.tensor_mul(gm2[:], packbuf[:, :, 1], packbuf[:, :, 1])
        nc.vector.tensor_sub(gvar[:], ex2[:], gm2[:])
        nc.scalar.activation(out=gvar[:], in_=gvar[:], func=AF.Sqrt,
                             bias=eps_t[:], scale=1.0)
        nc.vector.reciprocal(out=packbuf[:, :, 0], in_=gvar[:])
        scale_t, shift_t = [], []
        for ci in range(CT):
            bca = pstats.tile([128, 2], F32, name=f"bca{ci}_{rep}", tag="bca")
            nc.tensor.matmul(bca[:], b8_t[:], packbuf[:, ci, :], start=True, stop=True)
            sc = spool.tile([128, 1], F32, name=f"scale{ci}_{rep}", tag=f"scale{ci}")
            sh = spool.tile([128, 1], F32, name=f"shift{ci}_{rep}", tag=f"shift{ci}")
            tm = spool.tile([128, 1], F32, name=f"tmpm{ci}_{rep}", tag="tmpm")
            nc.vector.tensor_mul(sc[:], gnw_t[ci][:], bca[:, 0:1])
            nc.vector.tensor_mul(tm[:], bca[:, 1:2], sc[:])
            nc.vector.tensor_sub(sh[:], gnb_t[ci][:], tm[:])
            scale_t.append(sc)
            shift_t.append(sh)

        # ================= P3: fold GN into weights + bias folds ==============
        with tc.tile_pool(name=f"wfold_{rep}", bufs=1) as wfold:
            wq_s, wk_s, wv_s = [], [], []
            for nm, src, lst in (("wq", wqt_d, wq_s), ("wk", wkt_d, wk_s),
                                 ("wv", wvt_d, wv_s)):
                for ci in range(CT):
                    w = wfold.tile([128, C], F32R, name=f"{nm}{ci}_{rep}",
                                   tag=f"{nm}{ci}")
                    nc.sync.dma_start(w[:], src[ci])
                    lst.append(w)
            # bias folds with RAW weights: b' = b + w^T @ shift
            bq_sb, bv_sb = [], []
            for w_s, b_dram, lst, nm in ((wq_s, bq_d, bq_sb, "bq"),
                                         (wv_s, bv_d, bv_sb, "bv")):
                for co in range(CT):
                    pb = pstats.tile([128, 1], F32, name=f"pb{nm}{co}_{rep}", tag="pb")
                    for ci in range(CT):
                        nc.tensor.matmul(
                            pb[:],
                            w_s[ci][:].bitcast(F32)[:, co * 128:(co + 1) * 128],
                            shift_t[ci][:], start=(ci == 0), stop=(ci == CT - 1))
                    braw = spool.tile([128, 1], F32, name=f"{nm}r{co}_{rep}", tag="braw")
                    nc.sync.dma_start(braw[:], b_dram[co])
                    bt = spool.tile([128, 1], F32, name=f"{nm}f{co}_{rep}",
                                    tag=f"{nm}f{co}")
                    nc.vector.tensor_add(bt[:], pb[:], braw[:])
                    lst.append(bt)
            # bp' = bp + wp^T @ bv'
            bp_sb = []
            for co in range(CT):
                pb = pstats.tile([128, 1], F32, name=f"pbbp{co}_{rep}", tag="pb")
                for ci in range(CT):
                    nc.tensor.matmul(
                        pb[:], wpt_s[ci][:].bitcast(F32)[:, co * 128:(co + 1) * 128],
                        bv_sb[ci][:], start=(ci == 0), stop=(ci == CT - 1))
                braw = spool.tile([128, 1], F32, name=f"bpr{co}_{rep}", tag="braw")
                nc.sync.dma_start(braw[:], bp_d[co])
                bt = rpool.tile([128, 1], F32, name=f"bpf{co}_{rep}", tag=f"bpf{co}")
                nc.vector.tensor_add(bt[:], pb[:], braw[:])
                bp_sb.append(bt)
            # scale folds in place (f32 view in, f32r out = rounding write)
            for w_s in (wq_s, wk_s, wv_s):
                for ci in range(CT):
                    nc.vector.tensor_scalar_mul(out=w_s[ci][:],
                                                in0=w_s[ci][:].bitcast(F32),
                                                scalar1=scale_t[ci][:])

            # ================= P4: q / k / vT projections ====================
            with tc.tile_pool(name=f"pd_{rep}", bufs=5, space="PSUM") as pd:
                for sb in range(SBLK):
                    xb = [xpool.tile([128, 512], F32R, name=f"xc{sb}_{ci}_{rep}",
                                     tag=f"xb{ci}") for ci in range(CT)]
                    for ci in range(CT):
                        nc.sync.dma_start(xb[ci][:], x_d[ci, :, sb * 512:(sb + 1) * 512])
                    for co in range(CT):
                        pk = pd.tile([128, 512], F32, name=f"pk{sb}{co}_{rep}", tag="pd")
                        for ci in range(CT):
                            nc.tensor.matmul(pk[:],
                                             wk_s[ci][:, co * 128:(co + 1) * 128],
                                             xb[ci][:], start=(ci == 0),
                                             stop=(ci == CT - 1))
                        # k8[p] is [128, JT, 2, 128]: js blocks 4sb..4sb+3, slot t
                        nc.vector.tensor_copy(
                            k8_sb[co // 2][:, 4 * sb:4 * sb + 4, co % 2, :],
                            pk[:].rearrange("p (a b) -> p a b", a=4))
                    for js4 in range(4):
                        pv = pd.tile([128, 512], F32, name=f"pv{sb}{js4}_{rep}", tag="pd")
                        for ci in range(CT):
                            nc.tensor.matmul(pv[:],
                                             xb[ci][:, js4 * 128:(js4 + 1) * 128],
                                             wv_s[ci][:], start=(ci == 0),
                                             stop=(ci == CT - 1))
                        js = sb * 4 + js4
                        # vT8 is [128, JP, CT, 2, 128]: pair jp=js//2, slot js%2
                        nc.scalar.activation(
                            out=vT_sb[:, js // 2, :, js % 2, :],
                            in_=pv[:].rearrange("p (a b) -> p a b", a=4),
                            func=AF.Copy)
                    if sb < QBLK:
                        # columns [0, 2048) are this core's queries (host-permuted)
                        for co in range(CT):
                            pq = pd.tile([128, 512], F32, name=f"pq{sb}{co}_{rep}", tag="pd")
                            for ci in range(CT):
                                nc.tensor.matmul(pq[:],
                                                 wq_s[ci][:, co * 128:(co + 1) * 128],
                                                 xb[ci][:], start=(ci == 0),
                                                 stop=(ci == CT - 1))
                            # q8[p] is [128, IB, 2, IBW]: i-block sb, slot t
                            nc.vector.tensor_scalar(
                                out=q8_sb[co // 2][:, sb, co % 2, :],
                                in0=pq[:],
                                scalar1=bq_sb[co][:], scalar2=None, op0=OP.add)

    # ================= P5: attention + proj + residual =======================
    with tc.tile_pool(name=f"ex_{rep}", bufs=2) as expool, \
         tc.tile_pool(name=f"hn_{rep}", bufs=1) as hnpool, \
         tc.tile_pool(name=f"eo_{rep}", bufs=3) as eopool, \
         tc.tile_pool(name=f"psc_{rep}", bufs=3, space="PSUM") as psc, \
         tc.tile_pool(name=f"pph_{rep}", bufs=4, space="PSUM") as pph, \
         tc.tile_pool(name=f"psm_{rep}", bufs=1, space="PSUM") as psm:
        for ib in range(IB):
            isl = slice(ib * IBW, (ib + 1) * IBW)
            ex = expool.tile([128, JT, IBW], FP8E5, name=f"ex{ib}_{rep}", tag="ex")
            for js in range(JT):
                ps_ = psc.tile([128, IBW], F32, name=f"ps{ib}{js}_{rep}", tag="ps")
                for p in range(CP):
                    nc.tensor.matmul(ps_[:],
                                     k8_sb[p][:, js],
                                     q8_sb[p][:, ib], start=(p == 0),
                                     stop=(p == CP - 1), perf_mode=DRM)
                nc.scalar.activation(out=ex[:, js, :], in_=ps_[:], func=AF.Exp,
                                     scale=float(SCL))
            ph = [pph.tile([128, IBW], F32, name=f"ph{ib}{ci}_{rep}", tag="ph")
                  for ci in range(CT)]
            for jp in range(JP):
                for ci in range(CT):
                    nc.tensor.matmul(ph[ci][:],
                                     vT_sb[:, jp, ci],
                                     ex[:, 2 * jp:2 * jp + 2, :], start=(jp == 0),
                                     stop=(jp == JP - 1), skip_group_check=True,
                                     perf_mode=DRM)
            # denominator: ones-stationary DoubleRow matmul accumulated in PSUM
            pdn = psm.tile([128, IBW], F32, name=f"pdn{ib}_{rep}", tag="sm")
            for jp in range(JP):
                nc.tensor.matmul(pdn[:], on8_t[:],
                                 ex[:, 2 * jp:2 * jp + 2, :], start=(jp == 0),
                                 stop=(jp == JP - 1), skip_group_check=True,
                                 perf_mode=DRM)
            rbc = hnpool.tile([128, IBW], F32, name=f"rbc{ib}_{rep}", tag="rbc")
            nc.vector.reciprocal(out=rbc[:], in_=pdn[:])
            hs = hnpool.tile([128, CT, IBW], F32R, name=f"hs{ib}_{rep}", tag="hs")
            for ci in range(CT):
                nc.vector.tensor_mul(hs[:, ci, :], ph[ci][:], rbc[:])
            for co in range(CT):
                pp = psc.tile([128, IBW], F32, name=f"pp{ib}{co}_{rep}", tag="ps")
                for ci in range(CT):
                    nc.tensor.matmul(pp[:],
                                     wpt_s[ci][:, co * 128:(co + 1) * 128],
                                     hs[:, ci, :], start=(ci == 0),
                                     stop=(ci == CT - 1))
                xr = eopool.tile([128, IBW], F32, name=f"xr{ib}{co}_{rep}", tag="xr")
                nc.sync.dma_start(xr[:], x_d[co, :, isl].bitcast(F32))
                ot = eopool.tile([128, IBW], F32, name=f"ot{ib}{co}_{rep}", tag="ot")
                nc.vector.scalar_tensor_tensor(out=ot[:], in0=pp[:],
                                               scalar=bp_sb[co][:], in1=xr[:],
                                               op0=OP.add, op1=OP.add)
                nc.sync.dma_start(out_d[co, :, isl], ot[:])


# ---------------------------------------------------------------------------
# Host side
# ---------------------------------------------------------------------------
_NC_CACHE = {}


def _get_nc(reps=1):
    if reps not in _NC_CACHE:
        _NC_CACHE[reps] = build_nc(reps)
    return _NC_CACHE[reps]


def make_in_maps(x, gn_w, gn_b, wq, bq, wk, bk, wv, bv, wp, bp):
    xf = np.ascontiguousarray(np.asarray(x, dtype=np.float32)).reshape(B, C, S)
    g16 = np.zeros((128, 8), np.float32)
    g16[np.arange(128), np.arange(128) // 16] = 1.0
    b8 = np.ascontiguousarray(g16.T)
    shared = {
        "wqt": np.ascontiguousarray(np.asarray(wq, np.float32).T).reshape(CT, 128, C),
        "wkt": np.ascontiguousarray(np.asarray(wk, np.float32).T).reshape(CT, 128, C),
        "wvt": np.ascontiguousarray(np.asarray(wv, np.float32).T).reshape(CT, 128, C),
        "wpt": np.ascontiguousarray(np.asarray(wp, np.float32).T).reshape(CT, 128, C),
        "bq": np.asarray(bq, np.float32).reshape(CT, 128, 1),
        "bv": np.asarray(bv, np.float32).reshape(CT, 128, 1),
        "bp": np.asarray(bp, np.float32).reshape(CT, 128, 1),
        "gnw": np.asarray(gn_w, np.float32).reshape(CT, 128, 1),
        "gnb": np.asarray(gn_b, np.float32).reshape(CT, 128, 1),
        "g16": g16,
        "b8": b8,
        "on8": np.ones((128, 2, 128), ml_dtypes.float8_e5m2),
    }
    in_maps = []
    for core in range(NCORES):
        b, half = core // 2, core % 2
        xb = xf[b]
        if half == 0:
            xp = xb
        else:
            xp = np.concatenate([xb[:, HALF:], xb[:, :HALF]], axis=1)
        in_maps.append(dict(shared, x=np.ascontiguousarray(xp).reshape(CT, 128, S)))
    return in_maps


def assemble_out(results, H=64, W=64):
    out = np.empty((B, C, S), np.float32)
    for core in range(NCORES):
        b, half = core // 2, core % 2
        out[b][:, half * HALF:(half + 1) * HALF] = \
            results[core]["out"].reshape(C, HALF)
    return out.reshape(B, C, H, W)


def kernel(x, gn_w, gn_b, wq, bq, wk, bk, wv, bv, wp, bp, t1=64, t2=64):
    H, W = int(t1), int(t2)
    nc = _get_nc(1)
    in_maps = make_in_maps(x, gn_w, gn_b, wq, bq, wk, bk, wv, bv, wp, bp)
    res = run_bass_kernel_spmd(nc, in_maps, core_ids=list(range(NCORES)))
    return assemble_out(res.results, H, W)


# revision 31
# speedup vs baseline: 2.1047x; 1.1915x over previous
"""Trainium2 Bass kernel for an AttentionBlock (GroupNorm -> q/k/v 1x1 conv ->
full S x S attention -> proj 1x1 conv -> residual).

Problem shapes: x [4, 512, 64, 64] fp32, S = 4096 tokens, C = 512 channels,
GroupNorm with 32 groups of 16 channels.

Sharding: 8 cores = 4 batches x 2 query-halves. Core c handles batch c//2 and
query rows [half*2048, (half+1)*2048). Each core of a batch-pair redundantly
computes k/v for its batch (cheap vs attention) so no collectives are needed.

Math optimizations baked in:
  * GroupNorm is folded into the q/k/v weights: h = scale_c * x + shift_c with
    per-channel scale/shift derived from group stats, so
    q = (wq * scale) @ x + (bq + wq @ shift), and similarly k, v.
  * The bias folds run on folded weights via shiftd = shift/scale:
    (w*scale)^T @ shiftd == w^T @ shift, as [1, C] row matmuls (cheap on PE),
    then a DMA scatter turns rows into per-partition scalar columns.
  * k's bias term adds a per-query constant to every softmax row and cancels
    exactly -> never computed (bk unused).
  * v's bias adds bv' to the attention output (softmax rows sum to 1), folded
    into the proj bias: bp' = bp + wp @ (bv + wv @ shift).
  * Softmax runs without max-subtraction; probabilities are fp8e5m2 (wide
    exponent range covers both the O(e^8) peaks and the diffuse ~1e-4 tail),
    so scores (fp8e4m3 q/k) and attn@v / denominator run as DoubleRow fp8
    matmuls at >2x the bf16 PE rate (denominator = ones-stationary matmul).

Dtypes: q/k/v/proj projections in float32r (measured ~0.78 cyc/row); q/k/vT
as fp8e4m3; probabilities fp8e5m2; everything else fp32.

x stays resident in SBUF (64KB/partition) for stats, projections, and the
residual add -- loaded once with 8 big DMAs.

Layouts per core (partition dim first). DoubleRow operand slices MUST be
contiguous [p, 2, f] blocks (strided stationary slices crash the exec unit;
strided moving slices silently compute garbage):
  q8  [2][128, IB, 2, 512]     e4m3   scores DR moving     (slice [:, ib])
  k8  [2][128, JT, 2, 128]     e4m3   scores DR stationary (slice [:, js])
  vT8 [128, JP, CT, 2, 128]    e4m3   attn@v DR stationary (slice [:, jp, ci])
  ex8 [128, JT, 512]           e5m2   attn@v / den DR moving ([:, 2jp:2jp+2, :])
  scores^T [j, i]; attention output lands as h [c, i] which feeds proj.

P5 emission is software-pipelined: per i-block emit scores, then the PREVIOUS
block's proj/out (giving DVE time to produce hs), then denominator, then
attn@v.
"""

import numpy as np
import ml_dtypes

import concourse.bacc as bacc
import concourse.tile as tile
from concourse import mybir
from concourse.bass_utils import run_bass_kernel_spmd

F32 = mybir.dt.float32
F32R = mybir.dt.float32r
BF16 = mybir.dt.bfloat16
FP8E4 = mybir.dt.float8e4
FP8E5 = mybir.dt.float8e5
AF = mybir.ActivationFunctionType
OP = mybir.AluOpType
AX = mybir.AxisListType
DRM = mybir.MatmulPerfMode.DoubleRow

C = 512
S = 4096
B = 4
NCORES = 8
CT = 4          # channel tiles of 128
CP = 2          # channel pairs (DoubleRow k-tiles of 256)
SBLK = 8        # s-blocks of 512 for k/v
QBLK = 4        # q-blocks of 512 (half = 2048 columns)
IB = 4          # i-blocks of 512 for attention
IBW = 512
JT = 32         # j-tiles of 128
JP = 16         # j-pairs of 256 for DoubleRow
HALF = S // 2
EPS = 1e-5
GELEMS = 16 * S                      # elements per group (16 ch x 4096)
SCL = 1.0 / np.sqrt(np.float32(C))   # softmax scale


def build_nc(reps=1):
    """Build and compile the SPMD single-core program."""
    nc = bacc.Bacc("TRN2", target_bir_lowering=False, debug=False,
                   num_devices=NCORES)

    x_d = nc.dram_tensor("x", [CT, 128, S], F32R, kind="ExternalInput").ap()
    # weights host-packed as [128, CT, C] so each loads with ONE big DMA
    wqt_d = nc.dram_tensor("wqt", [128, CT, C], F32R, kind="ExternalInput").ap()
    wkt_d = nc.dram_tensor("wkt", [128, CT, C], F32R, kind="ExternalInput").ap()
    wvt_d = nc.dram_tensor("wvt", [128, CT, C], F32R, kind="ExternalInput").ap()
    wpt_d = nc.dram_tensor("wpt", [128, CT, C], F32R, kind="ExternalInput").ap()
    bqr_d = nc.dram_tensor("bqr", [1, C], F32, kind="ExternalInput").ap()
    bvr_d = nc.dram_tensor("bvr", [1, C], F32, kind="ExternalInput").ap()
    bpr_d = nc.dram_tensor("bpr", [1, C], F32, kind="ExternalInput").ap()
    # all small per-partition constants in ONE tensor: g16 (8) | gnw (4) | gnb (4)
    cst_d = nc.dram_tensor("cst", [128, 16], F32, kind="ExternalInput").ap()
    b8_d = nc.dram_tensor("b8", [8, 128], F32, kind="ExternalInput").ap()
    # out as [128, CT, HALF] so each i-block stores with ONE DMA
    out_d = nc.dram_tensor("out", [128, CT, HALF], F32, kind="ExternalOutput").ap()

    with tile.TileContext(nc) as tc:
        with tc.tile_pool(name="const", bufs=1) as cpool, \
             tc.tile_pool(name="resident", bufs=1) as rpool:
            # constants loaded once (2 DMAs + a memset'ed fp8 ones tile)
            cst_t = cpool.tile([128, 16], F32, name="cstt")
            b8_t = cpool.tile([8, 128], F32, name="b8t")
            on8_t = cpool.tile([128, 2, 128], FP8E5, name="on8t")
            eps_t = cpool.tile([8, 1], F32, name="epst")
            nc.sync.dma_start(cst_t[:], cst_d[:])
            nc.sync.dma_start(b8_t[:], b8_d[:])
            nc.vector.memset(on8_t[:], 1.0)
            nc.vector.memset(eps_t[:], EPS)
            g16_t = cst_t[:, 0:8]
            gnw_t = [cst_t[:, 8 + ci:9 + ci] for ci in range(CT)]
            gnb_t = [cst_t[:, 12 + ci:13 + ci] for ci in range(CT)]

            for rep in range(reps):
                emit_rep(nc, tc, rpool, rep,
                         x_d, wqt_d, wkt_d, wvt_d, wpt_d,
                         bqr_d, bvr_d, bpr_d,
                         g16_t, b8_t, on8_t, eps_t, gnw_t, gnb_t,
                         out_d)  # g16_t/gnw_t/gnb_t are APs into cst_t
    nc.compile()
    return nc


def emit_rep(nc, tc, rpool, rep, x_d, wqt_d, wkt_d, wvt_d, wpt_d,
             bqr_d, bvr_d, bpr_d, g16_t, b8_t, on8_t, eps_t,
             gnw_t, gnb_t, out_d):
    # ---- resident tensors (slots shared across reps via fixed tags) ----
    x_sb = [rpool.tile([128, S], F32R, name=f"x{ci}_{rep}", tag=f"x{ci}")
            for ci in range(CT)]
    k8_sb = [rpool.tile([128, JT, 2, 128], FP8E4, name=f"k8{p}_{rep}", tag=f"k8{p}")
             for p in range(CP)]
    q8_sb = [rpool.tile([128, IB, 2, IBW], FP8E4, name=f"q8{p}_{rep}", tag=f"q8{p}")
             for p in range(CP)]
    vT_sb = rpool.tile([128, JP, CT, 2, 128], FP8E4, name=f"vT_{rep}", tag="vT")
    # x FIRST on the DMA queue (stats gate everything); [128, 1024] chunks so
    # the per-chunk stats (and their sem acks) pipeline tightly behind the DMA
    NCH = 4
    CW = S // NCH
    for ci in range(CT):
        for h in range(NCH):
            nc.sync.dma_start(x_sb[ci][:, h * CW:(h + 1) * CW],
                              x_d[ci, :, h * CW:(h + 1) * CW])
    wpt_s = rpool.tile([128, CT, C], F32R, name=f"wp_{rep}", tag="wp")

    with tc.tile_pool(name=f"stat_{rep}", bufs=1) as spool, \
         tc.tile_pool(name=f"sscr_{rep}", bufs=2) as scrpool, \
         tc.tile_pool(name=f"wfold_{rep}", bufs=1) as wfold, \
         tc.tile_pool(name=f"pstat_{rep}", bufs=1, space="PSUM") as pstats:
        # raw weights: one DMA each, in first-use order (k, v, q, p)
        wk_t = wfold.tile([128, CT, C], F32R, name=f"wk_{rep}", tag="wk")
        wv_t = wfold.tile([128, CT, C], F32R, name=f"wv_{rep}", tag="wv")
        wq_t = wfold.tile([128, CT, C], F32R, name=f"wq_{rep}", tag="wq")
        nc.sync.dma_start(wk_t[:], wkt_d[:])
        nc.sync.dma_start(wv_t[:], wvt_d[:])
        nc.sync.dma_start(wq_t[:], wqt_d[:])
        nc.sync.dma_start(wpt_s[:], wpt_d[:])
        wk_s = [wk_t[:, ci, :] for ci in range(CT)]
        wv_s = [wv_t[:, ci, :] for ci in range(CT)]
        wq_s = [wq_t[:, ci, :] for ci in range(CT)]
        wp_s = [wpt_s[:, ci, :] for ci in range(CT)]
        # bias rows: tiny loads, issued early so they don't queue behind weights
        braw_t = {}
        for nm, d in (("bq", bqr_d), ("bv", bvr_d), ("bp", bpr_d)):
            br = spool.tile([1, C], F32, name=f"{nm}raw_{rep}", tag=f"{nm}raw")
            nc.sync.dma_start(br[:], d[:])
            braw_t[nm] = br

        # ====== P1: per-channel sum / sumsq over x (NCH chunks per ci) ========
        sums2 = spool.tile([128, CT, 2, NCH], F32, name=f"sums2_{rep}", tag="sums2")
        for ci in range(CT):
            for h in range(NCH):
                xf = x_sb[ci][:, h * CW:(h + 1) * CW].bitcast(F32)
                nc.vector.reduce_sum(out=sums2[:, ci, 0, h:h + 1], in_=xf,
                                     axis=AX.X)
                sq = scrpool.tile([128, CW], F32, name=f"sq{ci}{h}_{rep}",
                                  tag="sqscr")
                nc.scalar.activation(out=sq[:], in_=xf, func=AF.Square,
                                     accum_out=sums2[:, ci, 1, h:h + 1])

        # ================= P2: group stats -> per-channel scale/shift =========
        gpsum = pstats.tile([8, CT * 2 * NCH], F32, name=f"gps_{rep}", tag="g")
        for ci in range(CT):
            nc.tensor.matmul(gpsum[:, ci * 2 * NCH:(ci + 1) * 2 * NCH],
                             g16_t, sums2[:, ci, :, :], start=True, stop=True)
        # collapse the chunk axis: [8, CT, 2, NCH] -> [8, CT, 2]
        sq2 = spool.tile([8, CT, 2], F32, name=f"sq2_{rep}", tag="sq2")
        nc.vector.reduce_sum(
            out=sq2[:, :, :],
            in_=gpsum[:].rearrange("p (c t f) -> p c t f", t=2, f=NCH), axis=AX.X)
        packbuf = spool.tile([8, CT, 2], F32, name=f"pack_{rep}", tag="pack")
        ex2 = spool.tile([8, CT], F32, name=f"ex2_{rep}", tag="ex2")
        gm2 = spool.tile([8, CT], F32, name=f"gm2_{rep}", tag="gm2")
        gvar = spool.tile([8, CT], F32, name=f"gvar_{rep}", tag="gvar")
        nc.scalar.mul(out=packbuf[:, :, 1], in_=sq2[:, :, 0], mul=1.0 / GELEMS)
        nc.scalar.mul(out=ex2[:], in_=sq2[:, :, 1], mul=1.0 / GELEMS)
        nc.vector.tensor_mul(gm2[:], packbuf[:, :, 1], packbuf[:, :, 1])
        nc.vector.tensor_sub(gvar[:], ex2[:], gm2[:])
        nc.scalar.activation(out=gvar[:], in_=gvar[:], func=AF.Sqrt,
                             bias=eps_t[:], scale=1.0)
        nc.vector.reciprocal(out=packbuf[:, :, 0], in_=gvar[:])
        scale_t, shift_t, shiftd_t = [], [], []
        for ci in range(CT):
            bca = pstats.tile([128, 2], F32, name=f"bca{ci}_{rep}", tag="bca")
            nc.tensor.matmul(bca[:], b8_t[:], packbuf[:, ci, :], start=True, stop=True)
            sc = spool.tile([128, 1], F32, name=f"scale{ci}_{rep}", tag=f"scale{ci}")
            sh = spool.tile([128, 1], F32, name=f"shift{ci}_{rep}", tag=f"shift{ci}")
            # F32R: feeds f32r row-fold matmuls as stationary
            sd = spool.tile([128, 1], F32R, name=f"shiftd{ci}_{rep}", tag=f"shiftd{ci}")
            tm = spool.tile([128, 1], F32, name=f"tmpm{ci}_{rep}", tag="tmpm")
            nc.vector.tensor_mul(sc[:], gnw_t[ci], bca[:, 0:1])
            nc.vector.tensor_mul(tm[:], bca[:, 1:2], sc[:])
            nc.vector.tensor_sub(sh[:], gnb_t[ci], tm[:])
            # shiftd = shift / scale (for bias folds on folded weights)
            nc.vector.reciprocal(out=tm[:], in_=sc[:])
            nc.vector.tensor_mul(sd[:], sh[:], tm[:])
            scale_t.append(sc)
            shift_t.append(sh)
            shiftd_t.append(sd)

        # ================= P3: fold GN scale into weights (k first) ===========
        for w_s in (wk_s, wv_s, wq_s):
            for ci in range(CT):
                nc.vector.tensor_scalar_mul(out=w_s[ci],
                                            in0=w_s[ci].bitcast(F32),
                                            scalar1=scale_t[ci][:])

        # Row-form bias folds: b'^T = shiftd^T @ Wfold + b^T  (then scatter to
        # per-partition scalar columns via SBUF->SBUF DMA).
        def row_fold(nm, w_list, stat_list, out_cols, col_dt=F32):
            prow = pstats.tile([1, C], F32, name=f"prow{nm}_{rep}", tag="prow")
            for ci in range(CT):
                nc.tensor.matmul(prow[:], stat_list[ci][:], w_list[ci],
                                 start=(ci == 0), stop=(ci == CT - 1))
            brow = spool.tile([1, C], col_dt, name=f"{nm}row_{rep}", tag=f"{nm}row")
            nc.vector.tensor_add(brow[:], prow[:], braw_t[nm][:])
            for co in range(CT):
                nc.sync.dma_start(out_cols[co][:],
                                  brow[:, co * 128:(co + 1) * 128])

        bq_sb = [rpool.tile([128, 1], F32, name=f"bqc{co}_{rep}", tag=f"bqc{co}")
                 for co in range(CT)]
        bv_sb = [rpool.tile([128, 1], F32R, name=f"bvc{co}_{rep}", tag=f"bvc{co}")
                 for co in range(CT)]
        bp_sb = [rpool.tile([128, 1], F32, name=f"bpc{co}_{rep}", tag=f"bpc{co}")
                 for co in range(CT)]
        row_fold("bq", wq_s, shiftd_t, bq_sb)
        row_fold("bv", wv_s, shiftd_t, bv_sb, col_dt=F32R)
        row_fold("bp", wp_s, bv_sb, bp_sb)

        # ================= P4: q / k / vT projections =========================
        with tc.tile_pool(name=f"pd_{rep}", bufs=5, space="PSUM") as pd:
            for sb in range(SBLK):
                ssl = slice(sb * 512, (sb + 1) * 512)
                for co in range(CT):
                    pk = pd.tile([128, 512], F32, name=f"pk{sb}{co}_{rep}", tag="pd")
                    for ci in range(CT):
                        nc.tensor.matmul(pk[:],
                                         wk_s[ci][:, co * 128:(co + 1) * 128],
                                         x_sb[ci][:, ssl], start=(ci == 0),
                                         stop=(ci == CT - 1))
                    # k8[p] is [128, JT, 2, 128]: js blocks 4sb..4sb+3, slot t
                    kdst = k8_sb[co // 2][:, 4 * sb:4 * sb + 4, co % 2, :]
                    kin = pk[:].rearrange("p (a b) -> p a b", a=4)
                    if co < 2:
                        nc.vector.tensor_copy(kdst, kin)
                    else:
                        nc.scalar.activation(out=kdst, in_=kin, func=AF.Copy)
                for js4 in range(4):
                    pv = pd.tile([128, 512], F32, name=f"pv{sb}{js4}_{rep}", tag="pd")
                    for ci in range(CT):
                        nc.tensor.matmul(pv[:],
                                         x_sb[ci][:, sb * 512 + js4 * 128:
                                                  sb * 512 + (js4 + 1) * 128],
                                         wv_s[ci][:], start=(ci == 0),
                                         stop=(ci == CT - 1))
                    js = sb * 4 + js4
                    # vT8 is [128, JP, CT, 2, 128]: pair jp=js//2, slot js%2
                    vdst = vT_sb[:, js // 2, :, js % 2, :]
                    vin = pv[:].rearrange("p (a b) -> p a b", a=4)
                    if js4 < 2:
                        nc.scalar.activation(out=vdst, in_=vin, func=AF.Copy)
                    else:
                        nc.vector.tensor_copy(vdst, vin)
                if sb < QBLK:
                    # columns [0, 2048) are this core's queries (host-permuted)
                    for co in range(CT):
                        pq = pd.tile([128, 512], F32, name=f"pq{sb}{co}_{rep}",
                                     tag="pd")
                        for ci in range(CT):
                            nc.tensor.matmul(pq[:],
                                             wq_s[ci][:, co * 128:(co + 1) * 128],
                                             x_sb[ci][:, ssl], start=(ci == 0),
                                             stop=(ci == CT - 1))
                        # q8[p] is [128, IB, 2, IBW]: i-block sb, slot t
                        qdst = q8_sb[co // 2][:, sb, co % 2, :]
                        if co < 2:
                            nc.vector.tensor_scalar(
                                out=qdst, in0=pq[:], scalar1=bq_sb[co][:],
                                scalar2=None, op0=OP.add)
                        else:
                            nc.scalar.activation(out=qdst, in_=pq[:],
                                                 func=AF.Identity,
                                                 bias=bq_sb[co][:], scale=1.0)

    # ================= P5: attention + proj + residual =======================
    with tc.tile_pool(name=f"ex_{rep}", bufs=2) as expool, \
         tc.tile_pool(name=f"hn_{rep}", bufs=2) as hnpool, \
         tc.tile_pool(name=f"eo_{rep}", bufs=3) as eopool, \
         tc.tile_pool(name=f"psc_{rep}", bufs=3, space="PSUM") as psc, \
         tc.tile_pool(name=f"pph_{rep}", bufs=4, space="PSUM") as pph, \
         tc.tile_pool(name=f"psm_{rep}", bufs=1, space="PSUM") as psm:

        def emit_proj(ib, hs):
            isl = slice(ib * IBW, (ib + 1) * IBW)
            ot = eopool.tile([128, CT, IBW], F32, name=f"ot{ib}_{rep}", tag="ot")
            for co in range(CT):
                pp = psc.tile([128, IBW], F32, name=f"pp{ib}{co}_{rep}", tag="ps")
                for ci in range(CT):
                    nc.tensor.matmul(pp[:],
                                     wp_s[ci][:, co * 128:(co + 1) * 128],
                                     hs[:, ci, :], start=(ci == 0),
                                     stop=(ci == CT - 1))
                nc.vector.scalar_tensor_tensor(
                    out=ot[:, co, :], in0=pp[:], scalar=bp_sb[co][:],
                    in1=x_sb[co][:, isl].bitcast(F32), op0=OP.add, op1=OP.add)
            nc.sync.dma_start(out_d[:, :, isl], ot[:])

        prev = None
        for ib in range(IB):
            ex = expool.tile([128, JT, IBW], FP8E5, name=f"ex{ib}_{rep}", tag="ex")
            for js in range(JT):
                ps_ = psc.tile([128, IBW], F32, name=f"ps{ib}{js}_{rep}", tag="ps")
                for p in range(CP):
                    nc.tensor.matmul(ps_[:],
                                     k8_sb[p][:, js],
                                     q8_sb[p][:, ib], start=(p == 0),
                                     stop=(p == CP - 1), perf_mode=DRM)
                nc.scalar.activation(out=ex[:, js, :], in_=ps_[:], func=AF.Exp,
                                     scale=float(SCL))
            if prev is not None:
                emit_proj(*prev)
            # denominator: ones-stationary DoubleRow matmul accumulated in PSUM
            pdn = psm.tile([128, IBW], F32, name=f"pdn{ib}_{rep}", tag="sm")
            for jp in range(JP):
                nc.tensor.matmul(pdn[:], on8_t[:],
                                 ex[:, 2 * jp:2 * jp + 2, :], start=(jp == 0),
                                 stop=(jp == JP - 1), skip_group_check=True,
                                 perf_mode=DRM)
            ph = [pph.tile([128, IBW], F32, name=f"ph{ib}{ci}_{rep}", tag="ph")
                  for ci in range(CT)]
            for jp in range(JP):
                for ci in range(CT):
                    nc.tensor.matmul(ph[ci][:],
                                     vT_sb[:, jp, ci],
                                     ex[:, 2 * jp:2 * jp + 2, :], start=(jp == 0),
                                     stop=(jp == JP - 1), skip_group_check=True,
                                     perf_mode=DRM)
            rbc = hnpool.tile([128, IBW], F32, name=f"rbc{ib}_{rep}", tag="rbc")
            nc.vector.reciprocal(out=rbc[:], in_=pdn[:])
            hs = hnpool.tile([128, CT, IBW], F32R, name=f"hs{ib}_{rep}", tag="hs")
            for ci in range(CT):
                nc.vector.tensor_mul(hs[:, ci, :], ph[ci][:], rbc[:])
            prev = (ib, hs)
        emit_proj(*prev)


# ---------------------------------------------------------------------------
# Host side
# ---------------------------------------------------------------------------
_NC_CACHE = {}


def _get_nc(reps=1):
    if reps not in _NC_CACHE:
        _NC_CACHE[reps] = build_nc(reps)
    return _NC_CACHE[reps]


def make_in_maps(x, gn_w, gn_b, wq, bq, wk, bk, wv, bv, wp, bp):
    xf = np.ascontiguousarray(np.asarray(x, dtype=np.float32)).reshape(B, C, S)
    g16 = np.zeros((128, 8), np.float32)
    g16[np.arange(128), np.arange(128) // 16] = 1.0
    b8 = np.ascontiguousarray(g16.T)
    def packw(w):
        # w^T tiled [CT, 128, C] -> packed [128, CT, C]
        wt = np.asarray(w, np.float32).T.reshape(CT, 128, C)
        return np.ascontiguousarray(wt.transpose(1, 0, 2))

    shared = {
        "wqt": packw(wq),
        "wkt": packw(wk),
        "wvt": packw(wv),
        "wpt": packw(wp),
        "bqr": np.asarray(bq, np.float32).reshape(1, C),
        "bvr": np.asarray(bv, np.float32).reshape(1, C),
        "bpr": np.asarray(bp, np.float32).reshape(1, C),
        "cst": np.ascontiguousarray(np.concatenate(
            [g16,
             np.asarray(gn_w, np.float32).reshape(CT, 128).T,
             np.asarray(gn_b, np.float32).reshape(CT, 128).T], axis=1)),
        "b8": b8,
    }
    in_maps = []
    for core in range(NCORES):
        b, half = core // 2, core % 2
        xb = xf[b]
        if half == 0:
            xp = xb
        else:
            xp = np.concatenate([xb[:, HALF:], xb[:, :HALF]], axis=1)
        in_maps.append(dict(shared, x=np.ascontiguousarray(xp).reshape(CT, 128, S)))
    return in_maps


def assemble_out(results, H=64, W=64):
    out = np.empty((B, C, S), np.float32)
    for core in range(NCORES):
        b, half = core // 2, core % 2
        # device layout [128, CT, HALF] -> [C, HALF]
        arr = results[core]["out"].reshape(128, CT, HALF)
        out[b][:, half * HALF:(half + 1) * HALF] = \
            arr.transpose(1, 0, 2).reshape(C, HALF)
    return out.reshape(B, C, H, W)


def kernel(x, gn_w, gn_b, wq, bq, wk, bk, wv, bv, wp, bp, t1=64, t2=64):
    H, W = int(t1), int(t2)
    nc = _get_nc(1)
    in_maps = make_in_maps(x, gn_w, gn_b, wq, bq, wk, bk, wv, bv, wp, bp)
    res = run_bass_kernel_spmd(nc, in_maps, core_ids=list(range(NCORES)))
    return assemble_out(res.results, H, W)
